# revision 23
# baseline (speedup 1.0000x reference)
"""GroupedQueryAttention (B=2, S=2048, HID=2560, H=32, KV=8, D=80) on 8 NeuronCores.

Bass/Tile kernel, TP=4 over kv-head pairs x DP=2 over batch:
core c -> batch b = c//4, tp rank r = c%4 owning q heads [8r, 8r+8) and
kv heads [2r, 2r+2).

Final layout: projection groups (4 s-blocks = one 512-row q chunk) are
interleaved with attention chunks in program order
    G0 A0 G1 A1 G2 A2 G3 | A3
(A2 only needs k/v through G2 plus its own q, so it overlaps G3's
projection) letting the Tile scheduler fill PE stalls (softmax exp
waits) with projection matmuls so the HAM clock gate stays warm.
Key points vs the original baseline:
  - exp is batched: one ACT instruction covers 2 (A0-A2) or 3 (A3)
    128-k-blocks via multi-bank PSUM score tiles, cutting ACT
    instruction count and its 352-cycle/instr overhead.
  - two PSUM scopes: {psA 2x1 proj+transposes, psS 2x2 scores} while
    projection is alive, then {psX 2x3 batch-3 scores} for A3;
    psC (2x1) holds ctx accumulators and o_proj psum throughout.
  - softmax denominators: PV ones-columns (cols 80-96 of the 97-wide
    stationary) put the denom on aligned partition 96; DVE cross-
    partition copy -> reciprocal_approx_fast -> PE ones-matmul
    broadcast -> DVE multiply.  Nothing latency-critical sits on the
    gpsimd queue, because collectives occupy it for their whole
    duration.
  - RoPE runs on bf16 SBUF tiles with stride-0 broadcast APs reading
    the cos/sin tables directly (no staging DMAs); tables arrive
    host-pre-transposed in p-major layout (contiguous DMA).
  - out DMAs after each ReduceScatter are deferred to the next chunk's
    o_proj so their RS-completion waits never head-of-line-block a
    queue; the last chunk's RS is split into two 256-row halves.
"""

import math

import numpy as np
import ml_dtypes

B, S, HID = 2, 2048, 2560
H, KV, D = 32, 8, 80
NCORES = 8
TP, DP = 4, 2
NH = H // TP            # 8 q heads per core
NKV = KV // TP          # 2 kv heads per core
QF = NH * D             # 640
KF = NKV * D            # 160
QKVF = QF + 2 * KF      # 960
SB = S // 128           # 16 s-blocks
CHUNK = 512
QCN = S // CHUNK        # 4 q chunks
KT = HID // 128         # 20 contraction tiles
OKT = QF // 128         # 5 o_proj contraction tiles
ONC = HID // 512        # 5 o_proj col chunks
HALF = 480              # proj column split: [0:480) = q heads 0-5, [480:960)

NPBF16 = ml_dtypes.bfloat16

_NC = None


def _build_nc():
    import concourse.bass as bass
    import concourse.mybir as mybir
    import concourse.tile as tile
    from concourse import bacc
    from concourse.bass import ts

    f32 = mybir.dt.float32
    bf16 = mybir.dt.bfloat16
    MULT = mybir.AluOpType.mult
    ADD = mybir.AluOpType.add
    EXP = mybir.ActivationFunctionType.Exp
    COPY = mybir.ActivationFunctionType.Copy

    nc = bacc.Bacc(
        "TRN2",
        target_bir_lowering=False,
        debug=False,
        num_devices=NCORES,
    )
    xt_d = nc.declare_dram_parameter("xt", [HID, S], bf16, isOutput=False)
    wqkv_d = nc.declare_dram_parameter("wqkv", [HID, QKVF], bf16, isOutput=False)
    wo_d = nc.declare_dram_parameter("wo", [QF, HID], bf16, isOutput=False)
    cos8_d = nc.declare_dram_parameter("cos8", [128, SB * D], bf16, isOutput=False)
    ssin8_d = nc.declare_dram_parameter("ssin8", [128, SB * D], bf16, isOutput=False)
    tri_d = nc.declare_dram_parameter("tri", [128, 128], bf16, isOutput=False)
    ident_d = nc.declare_dram_parameter("ident", [128, 128], bf16, isOutput=False)
    out_d = nc.declare_dram_parameter("out", [QCN, 128, HID], bf16, isOutput=True)

    groups = [[0, 1, 2, 3], [4, 5, 6, 7]]
    scale = 1.0 / math.sqrt(D)

    with tile.TileContext(nc) as tc:
        with (
            tc.tile_pool(name="consts", bufs=1) as cp,
            tc.tile_pool(name="persist", bufs=1) as pp,
            tc.tile_pool(name="xtg", bufs=2) as xg,
            tc.tile_pool(name="qtg", bufs=2) as qg,
            tc.tile_pool(name="wqkvp", bufs=1) as wp,
            tc.tile_pool(name="wop", bufs=1) as wop,
            tc.tile_pool(name="ctxp", bufs=1) as ctxp,
            tc.tile_pool(name="ropep", bufs=2) as rp,
            tc.tile_pool(name="attn", bufs=2) as ap_,
            tc.tile_pool(name="dram", bufs=1, space="DRAM") as dp,
            tc.tile_pool(name="psC", bufs=2, space="PSUM") as psC,
        ):
            ident = cp.tile([128, 128], bf16)
            nc.scalar.dma_start(ident[:], ident_d[:])
            ones_row = cp.tile([1, 128], bf16)
            nc.gpsimd.memset(ones_row[:], 1.0)
            tri = cp.tile([128, 128], bf16)
            nc.scalar.dma_start(tri[:], tri_d[:])

            # k^T, v persistent for the whole kernel; q only per-group.
            kT_all = pp.tile([80, NKV, S], bf16)
            VW = 97  # per-kv slot: v (80 cols) | ones (cols 80-96); denom lands on aligned row 96
            v_all = pp.tile([128, SB, NKV * VW], bf16)
            nc.gpsimd.memset(v_all[:], 1.0)

            cosn = pp.tile([128, SB, D], bf16)
            nc.scalar.dma_start(cosn[:], cos8_d.rearrange("p (n d) -> p n d", n=SB))
            ssinn = pp.tile([128, SB, D], bf16)
            nc.scalar.dma_start(ssinn[:], ssin8_d.rearrange("p (n d) -> p n d", n=SB))

            # weights resident (wqkv DMAs issued interleaved with G0's xt below)
            wqkvs = [wp.tile([128, QKVF], bf16, name=f"w{k}", tag=f"w{k}")
                     for k in range(KT)]
            wos = []
            for kt in range(OKT):
                wo_k = wop.tile([128, HID], bf16, name=f"wo{kt}", tag=f"wo{kt}")
                nc.gpsimd.dma_start(wo_k[:], wo_d[ts(kt, 128), :])
                wos.append(wo_k)

            ctxP = [ctxp.tile([128, QCN, CHUNK], bf16, name=f"ctxP{kt}", tag=f"ctxP{kt}")
                    for kt in range(OKT)]

            o_part = [dp.tile([CHUNK, HID], bf16, name=f"opart{c}", tag=f"opart{c}")
                      for c in range(QCN)]
            rs_out = [dp.tile([128, HID], bf16, name=f"rs{c}", tag=f"rs{c}") for c in range(QCN)]

            # ---------------- projection of one 4-sb group ----------------
            qT_of_group = {}

            def proj_group(g, psA):
                # x^T slices for this group: 4 tiles of [128, 5kt, 512]
                xts = []
                for kk in range(4):
                    xt_t = xg.tile([128, 5, CHUNK], bf16, tag=f"xt{kk}")
                    for j in range(5):
                        k = 5 * kk + j
                        nc.sync.dma_start(
                            xt_t[:, j, :],
                            xt_d[ts(k, 128), g * CHUNK:(g + 1) * CHUNK])
                        if g == 0:  # first group: stream wqkv in alongside xt
                            nc.gpsimd.dma_start(wqkvs[k][:], wqkv_d[ts(k, 128), :])
                    xts.append(xt_t)

                qT = qg.tile([80, NH, CHUNK], bf16, tag="qT")
                qT_of_group[g] = qT

                pend = []  # (sb, rot) transposes deferred so they never
                           # head-of-line-block the PE queue behind DVE rope

                def do_transposes(sb, rot):
                    for bq in range(2):
                        ps_t = psA.tile([80, 512], bf16, tag="psA")
                        for j in range(4):
                            h = 4 * bq + j
                            nc.tensor.transpose(ps_t[:, ts(j, 128)], rot[:, ts(h, D)], ident[:])
                        nc.vector.tensor_copy(
                            qT[:, 4 * bq:4 * bq + 4, ts(sb % 4, 128)],
                            ps_t[:].rearrange("p (h s) -> p h s", h=4))
                    ps_t = psA.tile([80, 512], bf16, tag="psA")
                    for v in range(NKV):
                        nc.tensor.transpose(ps_t[:, ts(v, 128)], rot[:, QF + D * v:QF + D * (v + 1)], ident[:])
                    nc.vector.tensor_copy(
                        kT_all[:, :, ts(sb, 128)],
                        ps_t[:, 0:256].rearrange("p (v s) -> p v s", v=NKV))

                for si in range(4):
                    sb = 4 * g + si

                    rot = rp.tile([128, QKVF], bf16, tag="rot")  # rope'd q|k + raw v
                    for half in range(2):
                        c0 = HALF * half          # 0 or 480
                        c1 = HALF * (half + 1)    # 480 or 960
                        ps = psA.tile([128, HALF], f32, tag="psA")
                        for k in range(KT):
                            nc.tensor.matmul(
                                ps[:], xts[k // 5][:, k % 5, ts(si, 128)],
                                wqkvs[k][:, c0:c1],
                                start=(k == 0), stop=(k == KT - 1))
                        # cast to bf16 (ACT so DVE stays free; Copy is in every table set)
                        hb = rp.tile([128, HALF], bf16, tag=f"hb{half}")
                        nc.vector.tensor_copy(hb[:], ps[:])

                        # rope: half0 = 6 q heads; half1 = q6,q7,k0,k1 then v (raw)
                        nu = 6 if half == 0 else 4
                        w = nu * D  # rope'd width
                        h3 = hb[:, 0:w].rearrange("p (h d) -> p h d", h=nu)
                        c3 = cosn[:, sb:sb + 1, :].to_broadcast([128, nu, D])
                        s3a = ssinn[:, sb:sb + 1, 0:40].to_broadcast([128, nu, 40])
                        s3b = ssinn[:, sb:sb + 1, 40:80].to_broadcast([128, nu, 40])
                        t1 = rp.tile([128, 6 * D], bf16, tag="t1")
                        t2 = rp.tile([128, 6 * D], bf16, tag="t2")
                        t13 = t1[:, 0:w].rearrange("p (h d) -> p h d", h=nu)
                        nc.vector.tensor_tensor(t13[:], h3[:], c3, MULT)
                        t23 = t2[:, 0:w].rearrange("p (h d) -> p h d", h=nu)
                        nc.vector.tensor_tensor(t23[:, :, 0:40], h3[:, :, 40:80], s3a, MULT)
                        nc.vector.tensor_tensor(t23[:, :, 40:80], h3[:, :, 0:40], s3b, MULT)
                        nc.vector.tensor_tensor(rot[:, c0:c0 + w], t1[:, 0:w], t2[:, 0:w], ADD)
                        if half == 1:
                            # v: two heads' [*,80] -> v_all strided (ones col at 80)
                            nc.vector.tensor_copy(
                                v_all[:, sb, :].rearrange("p (v e) -> p v e", v=NKV)[:, :, 0:D],
                                hb[:, 320:480].rearrange("p (v d) -> p v d", v=NKV))

                    pend.append((sb, rot))
                    if len(pend) == 2 and si == 1:
                        pass  # flush below after next sb's proj is queued
                    if si >= 1 and len(pend) >= 2:
                        psb, prot = pend.pop(0)
                        do_transposes(psb, prot)
                for psb, prot in pend:
                    do_transposes(psb, prot)
                pend = []

            # ---------------- attention + o_proj + collective for one chunk ----
            pending_out = []

            def flush_out():
                while pending_out:
                    pqc = pending_out.pop(0)
                    nc.gpsimd.dma_start(out_d[pqc, :, :], rs_out[pqc][:])

            def attn_chunk(qc, last, spool, stag, bsz):
                qT = qT_of_group[qc]
                nkb = 4 * qc + 4
                kbgroups = []
                kb0 = 0
                while kb0 < nkb:
                    kbgroups.append(list(range(kb0, min(kb0 + bsz, nkb))))
                    kb0 += bsz
                for h in range(NH):
                    kv = h // (NH // NKV)
                    ctx_ps = psC.tile([128, CHUNK], f32, tag="psC")
                    for grp in kbgroups:
                        s_ps = spool.tile([128, bsz * CHUNK], f32, tag=stag)
                        offs = []
                        for j, kb in enumerate(grp):
                            jj = kb - 4 * qc
                            off = 128 * jj if jj > 0 else 0
                            offs.append(off)
                            nc.tensor.matmul(
                                s_ps[:, CHUNK * j + off:CHUNK * (j + 1)],
                                kT_all[:, kv, ts(kb, 128)],
                                qT[:, h, off:CHUNK],
                                start=True, stop=True)
                        ncol = CHUNK * len(grp)
                        p_t = ap_.tile([128, bsz * CHUNK], bf16, tag="p_t", bufs=3)
                        nc.scalar.activation(p_t[:, 0:ncol], s_ps[:, 0:ncol], EXP, scale=scale)
                        for j, kb in enumerate(grp):
                            jj = kb - 4 * qc
                            if jj >= 0:
                                o = CHUNK * j + 128 * jj
                                nc.vector.tensor_tensor(p_t[:, o:o + 128], p_t[:, o:o + 128], tri[:], MULT)
                        for j, kb in enumerate(grp):
                            nc.tensor.matmul(
                                ctx_ps[0:97, offs[j]:CHUNK],
                                v_all[:, kb, 97 * kv:97 * kv + 97],
                                p_t[:, CHUNK * j + offs[j]:CHUNK * (j + 1)],
                                start=(kb == 0), stop=(kb == nkb - 1))

                    # normalize: recip of denominator row, broadcast, multiply
                    dd = ap_.tile([1, CHUNK], f32, tag="dd", bufs=2)
                    nc.vector.tensor_copy(dd[0:1, :], ctx_ps[96:97, :])
                    drec = ap_.tile([1, CHUNK], f32, tag="drec", bufs=2)
                    nc.vector.reciprocal_approx_fast(out=drec[0:1, :], in_=dd[0:1, :])
                    drecb = ap_.tile([1, CHUNK], bf16, tag="drecb", bufs=2)
                    nc.vector.tensor_copy(drecb[0:1, :], drec[0:1, :])
                    rbc_ps = spool.tile([80, CHUNK], f32, tag=stag)
                    nc.tensor.matmul(rbc_ps[:], ones_row[0:1, 0:80], drecb[0:1, :],
                                     start=True, stop=True)
                    rbc = ap_.tile([80, CHUNK], bf16, tag="rbc", bufs=3)
                    nc.vector.tensor_copy(rbc[:], rbc_ps[:])
                    ctxn = ap_.tile([80, CHUNK], bf16, tag="ctxn", bufs=4)
                    nc.vector.tensor_tensor(ctxn[:], ctx_ps[0:80, :], rbc[:], MULT)
                    g0 = D * h
                    kt0, p0 = divmod(g0, 128)
                    n0 = min(D, 128 - p0)
                    nc.sync.dma_start(ctxP[kt0][p0:p0 + n0, qc, :], ctxn[0:n0, :])
                    if n0 < D:
                        nc.sync.dma_start(ctxP[kt0 + 1][0:D - n0, qc, :], ctxn[n0:D, :])

                flush_out()
                for i in range(4):
                    o_stage = ap_.tile([128, HID], bf16, tag="o_stage", bufs=2)
                    for n5 in range(ONC):
                        ps_o = psC.tile([128, 512], f32, tag="psC")
                        for kt in range(OKT):
                            nc.tensor.matmul(
                                ps_o[:], ctxP[kt][:, qc, ts(i, 128)],
                                wos[kt][:, ts(n5, 512)],
                                start=(kt == 0), stop=(kt == OKT - 1))
                        nc.vector.tensor_copy(o_stage[:, ts(n5, 512)], ps_o[:])
                    nc.sync.dma_start(o_part[qc][i * 128:(i + 1) * 128, :], o_stage[:])

                if not last:
                    nc.gpsimd.collective_compute(
                        "ReduceScatter",
                        mybir.AluOpType.add,
                        replica_groups=groups,
                        ins=[o_part[qc][:].opt()],
                        outs=[rs_out[qc][:].opt()],
                    )
                    pending_out.append(qc)
                else:
                    nc.gpsimd.collective_compute(
                        "ReduceScatter",
                        mybir.AluOpType.add,
                        replica_groups=groups,
                        ins=[o_part[qc][:].opt()],
                        outs=[rs_out[qc][:].opt()],
                    )
                    nc.gpsimd.dma_start(out_d[qc, :, :], rs_out[qc][:])
                    flush_out()

            # ------------- schedule: G0 A0 G1 A1 G2 A2 G3 | A3 -------------
            # (A2 only needs k/v through G2 and its own q, so it overlaps G3)
            with (
                tc.tile_pool(name="psA", bufs=2, space="PSUM") as psA,
                tc.tile_pool(name="psS", bufs=2, space="PSUM") as psS,
            ):
                proj_group(0, psA)
                attn_chunk(0, last=False, spool=psS, stag="psS", bsz=2)
                proj_group(1, psA)
                attn_chunk(1, last=False, spool=psS, stag="psS", bsz=2)
                proj_group(2, psA)
                attn_chunk(2, last=False, spool=psS, stag="psS", bsz=2)
                proj_group(3, psA)
            with tc.tile_pool(name="psX", bufs=3, space="PSUM") as psX:
                attn_chunk(3, last=True, spool=psX, stag="psX", bsz=2)

    nc.compile()
    return nc


def get_nc():
    global _NC
    if _NC is None:
        _NC = _build_nc()
    return _NC


def make_in_maps(hidden_states, cos_freqs, sin_freqs, Wq, Wk, Wv, Wo):
    f32 = np.float32
    x = np.asarray(hidden_states, f32)
    cos = np.asarray(cos_freqs, f32)
    sin = np.asarray(sin_freqs, f32)
    Wq = np.asarray(Wq, f32)
    Wk = np.asarray(Wk, f32)
    Wv = np.asarray(Wv, f32)
    Wo = np.asarray(Wo, f32)

    xt = [np.ascontiguousarray(x[b].T).astype(NPBF16) for b in range(B)]
    ssin = np.concatenate([-sin[:, :D // 2], sin[:, D // 2:]], axis=1)
    # p-major layout [128, SB*D]: row p holds cos for s = n*128 + p, n = 0..SB-1
    cos8 = np.ascontiguousarray(
        cos.reshape(SB, 128, D).transpose(1, 0, 2).reshape(128, SB * D)).astype(NPBF16)
    ssin8 = np.ascontiguousarray(
        ssin.reshape(SB, 128, D).transpose(1, 0, 2).reshape(128, SB * D)).astype(NPBF16)
    tri = np.triu(np.ones((128, 128), f32)).astype(NPBF16)
    ident = np.eye(128, dtype=f32).astype(NPBF16)

    in_maps = []
    for c in range(NCORES):
        b, r = divmod(c, TP)
        wqkv = np.concatenate([
            Wq[:, QF * r:QF * (r + 1)],
            Wk[:, KF * r:KF * (r + 1)],
            Wv[:, KF * r:KF * (r + 1)],
        ], axis=1).astype(NPBF16)
        wo = Wo[QF * r:QF * (r + 1), :].astype(NPBF16)
        in_maps.append({
            "xt": xt[b], "wqkv": wqkv, "wo": wo,
            "cos8": cos8, "ssin8": ssin8, "tri": tri, "ident": ident,
        })
    return in_maps


def assemble_out(results):
    out = np.empty((B, S, HID), np.float32)
    for c in range(NCORES):
        b, r = divmod(c, TP)
        shard = np.asarray(results[c]["out"]).astype(np.float32)  # [QCN, 128, HID]
        for qc in range(QCN):
            out[b, qc * CHUNK + r * 128:qc * CHUNK + (r + 1) * 128, :] = shard[qc]
    return out


def kernel(hidden_states, cos_freqs, sin_freqs, Wq, Wk, Wv, Wo):
    from concourse.bass_utils import run_bass_kernel_spmd

    nc = get_nc()
    in_maps = make_in_maps(hidden_states, cos_freqs, sin_freqs, Wq, Wk, Wv, Wo)
    res = run_bass_kernel_spmd(nc, in_maps, list(range(NCORES)))
    return assemble_out(res.results)


# revision 28
# speedup vs baseline: 1.0237x; 1.0237x over previous
"""GroupedQueryAttention (B=2, S=2048, HID=2560, H=32, KV=8, D=80) on 8 NeuronCores.

Bass/Tile kernel, TP=4 over kv-head pairs x DP=2 over batch:
core c -> batch b = c//4, tp rank r = c%4 owning q heads [8r, 8r+8) and
kv heads [2r, 2r+2).

Final layout: projection groups (4 s-blocks = one 512-row q chunk) are
interleaved with attention chunks in program order
    G0 A0 G1 A1 G2 A2 G3 | A3
(A2 only needs k/v through G2 plus its own q, so it overlaps G3's
projection) letting the Tile scheduler fill PE stalls (softmax exp
waits) with projection matmuls so the HAM clock gate stays warm.
Key points vs the original baseline:
  - exp is batched: one ACT instruction covers 2 (A0-A2) or 3 (A3)
    128-k-blocks via multi-bank PSUM score tiles, cutting ACT
    instruction count and its 352-cycle/instr overhead.
  - two PSUM scopes: {psA 2x1 proj+transposes, psS 2x2 scores} while
    projection is alive, then {psX 2x3 batch-3 scores} for A3;
    psC (2x1) holds ctx accumulators and o_proj psum throughout.
  - softmax denominators: PV ones-columns (cols 80-96 of the 97-wide
    stationary) put the denom on aligned partition 96; DVE cross-
    partition copy -> reciprocal_approx_fast -> PE ones-matmul
    broadcast -> DVE multiply.  Nothing latency-critical sits on the
    gpsimd queue, because collectives occupy it for their whole
    duration.
  - RoPE runs on bf16 SBUF tiles with stride-0 broadcast APs reading
    the cos/sin tables directly (no staging DMAs); tables arrive
    host-pre-transposed in p-major layout (contiguous DMA).
  - out DMAs after each ReduceScatter are deferred to the next chunk's
    o_proj so their RS-completion waits never head-of-line-block a
    queue; the last chunk's RS is split into two 256-row halves.
"""

import math

import numpy as np
import ml_dtypes

B, S, HID = 2, 2048, 2560
H, KV, D = 32, 8, 80
NCORES = 8
TP, DP = 4, 2
NH = H // TP            # 8 q heads per core
NKV = KV // TP          # 2 kv heads per core
QF = NH * D             # 640
KF = NKV * D            # 160
QKVF = QF + 2 * KF      # 960
SB = S // 128           # 16 s-blocks
CHUNK = 512
QCN = S // CHUNK        # 4 q chunks
KT = HID // 128         # 20 contraction tiles
OKT = QF // 128         # 5 o_proj contraction tiles
ONC = HID // 512        # 5 o_proj col chunks
HALF = 480              # proj column split: [0:480) = q heads 0-5, [480:960)

NPBF16 = ml_dtypes.bfloat16

_NC = None


def _build_nc():
    import concourse.bass as bass
    import concourse.mybir as mybir
    import concourse.tile as tile
    from concourse import bacc
    from concourse.bass import ts

    f32 = mybir.dt.float32
    bf16 = mybir.dt.bfloat16
    MULT = mybir.AluOpType.mult
    ADD = mybir.AluOpType.add
    EXP = mybir.ActivationFunctionType.Exp
    COPY = mybir.ActivationFunctionType.Copy

    nc = bacc.Bacc(
        "TRN2",
        target_bir_lowering=False,
        debug=False,
        num_devices=NCORES,
    )
    xt_d = nc.declare_dram_parameter("xt", [HID, S], bf16, isOutput=False)
    wqkv_d = nc.declare_dram_parameter("wqkv", [HID, QKVF], bf16, isOutput=False)
    wo_d = nc.declare_dram_parameter("wo", [QF, HID], bf16, isOutput=False)
    cos8_d = nc.declare_dram_parameter("cos8", [128, SB * D], bf16, isOutput=False)
    ssin8_d = nc.declare_dram_parameter("ssin8", [128, SB * D], bf16, isOutput=False)
    tri_d = nc.declare_dram_parameter("tri", [128, 128], bf16, isOutput=False)
    ident_d = nc.declare_dram_parameter("ident", [128, 128], bf16, isOutput=False)
    out_d = nc.declare_dram_parameter("out", [QCN, 128, HID], bf16, isOutput=True)

    groups = [[0, 1, 2, 3], [4, 5, 6, 7]]
    scale = 1.0 / math.sqrt(D)

    with tile.TileContext(nc) as tc:
        with (
            tc.tile_pool(name="consts", bufs=1) as cp,
            tc.tile_pool(name="persist", bufs=1) as pp,
            tc.tile_pool(name="xtg", bufs=2) as xg,
            tc.tile_pool(name="qtg", bufs=2) as qg,
            tc.tile_pool(name="wqkvp", bufs=1) as wp,
            tc.tile_pool(name="wop", bufs=1) as wop,
            tc.tile_pool(name="ctxp", bufs=1) as ctxp,
            tc.tile_pool(name="ropep", bufs=2) as rp,
            tc.tile_pool(name="attn", bufs=2) as ap_,
            tc.tile_pool(name="dram", bufs=1, space="DRAM") as dp,
            tc.tile_pool(name="psC", bufs=2, space="PSUM") as psC,
        ):
            ident = cp.tile([128, 128], bf16)
            nc.scalar.dma_start(ident[:], ident_d[:])
            ones_row = cp.tile([1, 128], bf16)
            nc.gpsimd.memset(ones_row[:], 1.0)
            tri = cp.tile([128, 128], bf16)
            nc.scalar.dma_start(tri[:], tri_d[:])

            # k^T, v persistent for the whole kernel; q only per-group.
            kT_all = pp.tile([80, NKV, S], bf16)
            VW = 97  # per-kv slot: v (80 cols) | ones (cols 80-96); denom lands on aligned row 96
            v_all = pp.tile([128, SB, NKV * VW], bf16)
            nc.gpsimd.memset(v_all[:], 1.0)

            cosn = pp.tile([128, SB, D], bf16)
            nc.scalar.dma_start(cosn[:], cos8_d.rearrange("p (n d) -> p n d", n=SB))
            ssinn = pp.tile([128, SB, D], bf16)
            nc.scalar.dma_start(ssinn[:], ssin8_d.rearrange("p (n d) -> p n d", n=SB))

            # weights resident (wqkv DMAs issued interleaved with G0's xt below)
            wqkvs = [wp.tile([128, QKVF], bf16, name=f"w{k}", tag=f"w{k}")
                     for k in range(KT)]
            wos = []
            for kt in range(OKT):
                wo_k = wop.tile([128, HID], bf16, name=f"wo{kt}", tag=f"wo{kt}")
                nc.gpsimd.dma_start(wo_k[:], wo_d[ts(kt, 128), :])
                wos.append(wo_k)

            ctxP = [ctxp.tile([128, QCN, CHUNK], bf16, name=f"ctxP{kt}", tag=f"ctxP{kt}")
                    for kt in range(OKT)]

            o_part = [dp.tile([CHUNK, HID], bf16, name=f"opart{c}", tag=f"opart{c}")
                      for c in range(QCN)]
            rs_out = [dp.tile([128, HID], bf16, name=f"rs{c}", tag=f"rs{c}") for c in range(QCN)]

            # ---------------- projection of one 4-sb group ----------------
            qT_of_group = {}

            def proj_group(g, psA):
                # x^T slices for this group: 4 tiles of [128, 5kt, 512]
                xts = []
                for kk in range(4):
                    xt_t = xg.tile([128, 5, CHUNK], bf16, tag=f"xt{kk}")
                    for j in range(5):
                        k = 5 * kk + j
                        nc.sync.dma_start(
                            xt_t[:, j, :],
                            xt_d[ts(k, 128), g * CHUNK:(g + 1) * CHUNK])
                        if g == 0:  # first group: stream wqkv in alongside xt
                            nc.scalar.dma_start(wqkvs[k][:], wqkv_d[ts(k, 128), :])
                    xts.append(xt_t)

                qT = qg.tile([80, NH, CHUNK], bf16, tag="qT")
                qT_of_group[g] = qT

                pend = []  # (sb, rot) transposes deferred so they never
                           # head-of-line-block the PE queue behind DVE rope

                def do_transposes(sb, rot):
                    for bq in range(2):
                        ps_t = psA.tile([80, 512], bf16, tag="psA")
                        for j in range(4):
                            h = 4 * bq + j
                            nc.tensor.transpose(ps_t[:, ts(j, 128)], rot[:, ts(h, D)], ident[:])
                        nc.vector.tensor_copy(
                            qT[:, 4 * bq:4 * bq + 4, ts(sb % 4, 128)],
                            ps_t[:].rearrange("p (h s) -> p h s", h=4))
                    ps_t = psA.tile([80, 512], bf16, tag="psA")
                    for v in range(NKV):
                        nc.tensor.transpose(ps_t[:, ts(v, 128)], rot[:, QF + D * v:QF + D * (v + 1)], ident[:])
                    nc.vector.tensor_copy(
                        kT_all[:, :, ts(sb, 128)],
                        ps_t[:, 0:256].rearrange("p (v s) -> p v s", v=NKV))

                for si in range(4):
                    sb = 4 * g + si

                    rot = rp.tile([128, QKVF], bf16, tag="rot")  # rope'd q|k + raw v
                    for half in range(2):
                        c0 = HALF * half          # 0 or 480
                        c1 = HALF * (half + 1)    # 480 or 960
                        ps = psA.tile([128, HALF], f32, tag="psA")
                        for k in range(KT):
                            nc.tensor.matmul(
                                ps[:], xts[k // 5][:, k % 5, ts(si, 128)],
                                wqkvs[k][:, c0:c1],
                                start=(k == 0), stop=(k == KT - 1))
                        # cast to bf16 (ACT so DVE stays free; Copy is in every table set)
                        hb = rp.tile([128, HALF], bf16, tag=f"hb{half}")
                        nc.vector.tensor_copy(hb[:], ps[:])

                        # rope: half0 = 6 q heads; half1 = q6,q7,k0,k1 then v (raw)
                        nu = 6 if half == 0 else 4
                        w = nu * D  # rope'd width
                        h3 = hb[:, 0:w].rearrange("p (h d) -> p h d", h=nu)
                        c3 = cosn[:, sb:sb + 1, :].to_broadcast([128, nu, D])
                        s3a = ssinn[:, sb:sb + 1, 0:40].to_broadcast([128, nu, 40])
                        s3b = ssinn[:, sb:sb + 1, 40:80].to_broadcast([128, nu, 40])
                        t1 = rp.tile([128, 6 * D], bf16, tag="t1")
                        t2 = rp.tile([128, 6 * D], bf16, tag="t2")
                        t13 = t1[:, 0:w].rearrange("p (h d) -> p h d", h=nu)
                        nc.vector.tensor_tensor(t13[:], h3[:], c3, MULT)
                        t23 = t2[:, 0:w].rearrange("p (h d) -> p h d", h=nu)
                        nc.vector.tensor_tensor(t23[:, :, 0:40], h3[:, :, 40:80], s3a, MULT)
                        nc.vector.tensor_tensor(t23[:, :, 40:80], h3[:, :, 0:40], s3b, MULT)
                        nc.vector.tensor_tensor(rot[:, c0:c0 + w], t1[:, 0:w], t2[:, 0:w], ADD)
                        if half == 1:
                            # v: two heads' [*,80] -> v_all strided (ones col at 80)
                            nc.vector.tensor_copy(
                                v_all[:, sb, :].rearrange("p (v e) -> p v e", v=NKV)[:, :, 0:D],
                                hb[:, 320:480].rearrange("p (v d) -> p v d", v=NKV))

                    pend.append((sb, rot))
                    if len(pend) == 2 and si == 1:
                        pass  # flush below after next sb's proj is queued
                    if si >= 1 and len(pend) >= 2:
                        psb, prot = pend.pop(0)
                        do_transposes(psb, prot)
                for psb, prot in pend:
                    do_transposes(psb, prot)
                pend = []

            # ---------------- attention + o_proj + collective for one chunk ----
            pending_out = []

            def flush_out():
                while pending_out:
                    pqc = pending_out.pop(0)
                    nc.gpsimd.dma_start(out_d[pqc, :, :], rs_out[pqc][:])

            def attn_chunk(qc, last, spool, stag, bsz):
                qT = qT_of_group[qc]
                nkb = 4 * qc + 4
                kbgroups = []
                kb0 = 0
                while kb0 < nkb:
                    kbgroups.append(list(range(kb0, min(kb0 + bsz, nkb))))
                    kb0 += bsz
                for h in range(NH):
                    kv = h // (NH // NKV)
                    ctx_ps = psC.tile([128, CHUNK], f32, tag="psC")
                    for grp in kbgroups:
                        s_ps = spool.tile([128, bsz * CHUNK], f32, tag=stag)
                        offs = []
                        for j, kb in enumerate(grp):
                            jj = kb - 4 * qc
                            off = 128 * jj if jj > 0 else 0
                            offs.append(off)
                            nc.tensor.matmul(
                                s_ps[:, CHUNK * j + off:CHUNK * (j + 1)],
                                kT_all[:, kv, ts(kb, 128)],
                                qT[:, h, off:CHUNK],
                                start=True, stop=True)
                        ncol = CHUNK * len(grp)
                        p_t = ap_.tile([128, bsz * CHUNK], bf16, tag="p_t", bufs=4)
                        nc.scalar.activation(p_t[:, 0:ncol], s_ps[:, 0:ncol], EXP, scale=scale)
                        for j, kb in enumerate(grp):
                            jj = kb - 4 * qc
                            if jj >= 0:
                                o = CHUNK * j + 128 * jj
                                nc.vector.tensor_tensor(p_t[:, o:o + 128], p_t[:, o:o + 128], tri[:], MULT)
                        for j, kb in enumerate(grp):
                            nc.tensor.matmul(
                                ctx_ps[0:97, offs[j]:CHUNK],
                                v_all[:, kb, 97 * kv:97 * kv + 97],
                                p_t[:, CHUNK * j + offs[j]:CHUNK * (j + 1)],
                                start=(kb == 0), stop=(kb == nkb - 1))

                    # normalize: recip of denominator row, broadcast, multiply
                    dd = ap_.tile([1, CHUNK], f32, tag="dd", bufs=2)
                    nc.vector.tensor_copy(dd[0:1, :], ctx_ps[96:97, :])
                    drec = ap_.tile([1, CHUNK], f32, tag="drec", bufs=2)
                    nc.vector.reciprocal_approx_fast(out=drec[0:1, :], in_=dd[0:1, :])
                    drecb = ap_.tile([1, CHUNK], bf16, tag="drecb", bufs=2)
                    nc.vector.tensor_copy(drecb[0:1, :], drec[0:1, :])
                    rbc_ps = spool.tile([80, CHUNK], f32, tag=stag)
                    nc.tensor.matmul(rbc_ps[:], ones_row[0:1, 0:80], drecb[0:1, :],
                                     start=True, stop=True)
                    rbc = ap_.tile([80, CHUNK], bf16, tag="rbc", bufs=3)
                    nc.vector.tensor_copy(rbc[:], rbc_ps[:])
                    ctxn = ap_.tile([80, CHUNK], bf16, tag="ctxn", bufs=4)
                    nc.vector.tensor_tensor(ctxn[:], ctx_ps[0:80, :], rbc[:], MULT)
                    g0 = D * h
                    kt0, p0 = divmod(g0, 128)
                    n0 = min(D, 128 - p0)
                    nc.sync.dma_start(ctxP[kt0][p0:p0 + n0, qc, :], ctxn[0:n0, :])
                    if n0 < D:
                        nc.sync.dma_start(ctxP[kt0 + 1][0:D - n0, qc, :], ctxn[n0:D, :])

                flush_out()
                for i in range(4):
                    o_stage = ap_.tile([128, HID], bf16, tag="o_stage", bufs=2)
                    for n5 in range(ONC):
                        ps_o = psC.tile([128, 512], f32, tag="psC")
                        for kt in range(OKT):
                            nc.tensor.matmul(
                                ps_o[:], ctxP[kt][:, qc, ts(i, 128)],
                                wos[kt][:, ts(n5, 512)],
                                start=(kt == 0), stop=(kt == OKT - 1))
                        nc.vector.tensor_copy(o_stage[:, ts(n5, 512)], ps_o[:])
                    nc.sync.dma_start(o_part[qc][i * 128:(i + 1) * 128, :], o_stage[:])

                if not last:
                    nc.gpsimd.collective_compute(
                        "ReduceScatter",
                        mybir.AluOpType.add,
                        replica_groups=groups,
                        ins=[o_part[qc][:].opt()],
                        outs=[rs_out[qc][:].opt()],
                    )
                    pending_out.append(qc)
                else:
                    # split by o_proj i-block pairs so the first RS overlaps
                    # the second half's o_proj
                    for r0, r1 in ((0, 256), (256, 512)):
                        nc.gpsimd.collective_compute(
                            "ReduceScatter",
                            mybir.AluOpType.add,
                            replica_groups=groups,
                            ins=[o_part[qc][r0:r1, :].opt()],
                            outs=[rs_out[qc][r0 // 4:r1 // 4, :].opt()],
                        )
                        nc.gpsimd.dma_start(
                            out_d[qc, r0 // 4:r1 // 4, :],
                            rs_out[qc][r0 // 4:r1 // 4, :])
                    flush_out()

            # ------------- schedule: G0 A0 G1 A1 G2 A2 G3 | A3 -------------
            # (A2 only needs k/v through G2 and its own q, so it overlaps G3)
            with (
                tc.tile_pool(name="psA", bufs=2, space="PSUM") as psA,
                tc.tile_pool(name="psS", bufs=2, space="PSUM") as psS,
            ):
                proj_group(0, psA)
                attn_chunk(0, last=False, spool=psS, stag="psS", bsz=2)
                proj_group(1, psA)
                attn_chunk(1, last=False, spool=psS, stag="psS", bsz=2)
                proj_group(2, psA)
                attn_chunk(2, last=False, spool=psS, stag="psS", bsz=2)
                proj_group(3, psA)
            with tc.tile_pool(name="psX", bufs=3, space="PSUM") as psX:
                attn_chunk(3, last=True, spool=psX, stag="psX", bsz=2)

    nc.compile()
    return nc


def get_nc():
    global _NC
    if _NC is None:
        _NC = _build_nc()
    return _NC


def make_in_maps(hidden_states, cos_freqs, sin_freqs, Wq, Wk, Wv, Wo):
    f32 = np.float32
    x = np.asarray(hidden_states, f32)
    cos = np.asarray(cos_freqs, f32)
    sin = np.asarray(sin_freqs, f32)
    Wq = np.asarray(Wq, f32)
    Wk = np.asarray(Wk, f32)
    Wv = np.asarray(Wv, f32)
    Wo = np.asarray(Wo, f32)

    xt = [np.ascontiguousarray(x[b].T).astype(NPBF16) for b in range(B)]
    ssin = np.concatenate([-sin[:, :D // 2], sin[:, D // 2:]], axis=1)
    # p-major layout [128, SB*D]: row p holds cos for s = n*128 + p, n = 0..SB-1
    cos8 = np.ascontiguousarray(
        cos.reshape(SB, 128, D).transpose(1, 0, 2).reshape(128, SB * D)).astype(NPBF16)
    ssin8 = np.ascontiguousarray(
        ssin.reshape(SB, 128, D).transpose(1, 0, 2).reshape(128, SB * D)).astype(NPBF16)
    tri = np.triu(np.ones((128, 128), f32)).astype(NPBF16)
    ident = np.eye(128, dtype=f32).astype(NPBF16)

    in_maps = []
    for c in range(NCORES):
        b, r = divmod(c, TP)
        wqkv = np.concatenate([
            Wq[:, QF * r:QF * (r + 1)],
            Wk[:, KF * r:KF * (r + 1)],
            Wv[:, KF * r:KF * (r + 1)],
        ], axis=1).astype(NPBF16)
        wo = Wo[QF * r:QF * (r + 1), :].astype(NPBF16)
        in_maps.append({
            "xt": xt[b], "wqkv": wqkv, "wo": wo,
            "cos8": cos8, "ssin8": ssin8, "tri": tri, "ident": ident,
        })
    return in_maps


def assemble_out(results):
    out = np.empty((B, S, HID), np.float32)
    LAST = 3  # chunk executed last, reduce-scattered in two 256-row halves
    for c in range(NCORES):
        b, r = divmod(c, TP)
        shard = np.asarray(results[c]["out"]).astype(np.float32)  # [QCN, 128, HID]
        for qc in range(QCN):
            if qc != LAST:
                out[b, qc * CHUNK + r * 128:qc * CHUNK + (r + 1) * 128, :] = shard[qc]
            else:
                for hh in range(2):
                    base = qc * CHUNK + 256 * hh + 64 * r
                    out[b, base:base + 64, :] = shard[qc, 64 * hh:64 * hh + 64]
    return out


def kernel(hidden_states, cos_freqs, sin_freqs, Wq, Wk, Wv, Wo):
    from concourse.bass_utils import run_bass_kernel_spmd

    nc = get_nc()
    in_maps = make_in_maps(hidden_states, cos_freqs, sin_freqs, Wq, Wk, Wv, Wo)
    res = run_bass_kernel_spmd(nc, in_maps, list(range(NCORES)))
    return assemble_out(res.results)
